# revision 1
# baseline (speedup 1.0000x reference)
"""3-layer LSTM decoder (T=256, B=1024, H=64/128/1) with locked dropout.

Data-parallel over batch: B=1024 -> 128 per core x 8 NeuronCores.
Single fused Bass/Tile kernel per core runs all three layer scans as a
wavefront (iteration tau computes L1 step tau, L2 step tau-1, L3 step
tau-2) so the per-step work of all layers batches into a few wide
engine instructions:

  - 16 matmuls/iter accumulate gate pre-activations into one PSUM tile
    of 8 slots [i2|i13|f2|f13|o2|o13|g2|g13] (2 = layer2; 13 = layer1
    at partitions 0-63 + layer3 at partition 64). Input GEMMs and
    biases are folded in as extra contraction rows (x row, ones row).
  - ONE sigmoid ACT over the whole 1024-wide PSUM rect per iteration.
    tanh is computed as 2*sigmoid(2x)-1: g-gate weights are pre-scaled
    by 2 and the cell state is tracked as C=c/2, hidden as H=h/2, with
    the compensating 2x folded into all weights that consume H.
  - Cell update in 3 packed DVE ops over [128,256]:
      u = (sg-0.5)*si ; v = sf*C ; C = v+u
    then ONE sigmoid ACT S=sigma(4C) (=sigma(2c)) and H=(S-0.5)*so.
  - Locked dropout masks are applied to H with small per-iter DVE ops;
    mask3 (and the final 2x for h3=2*H3) is applied on the host during
    the gather, as is the output assembly.
"""

import os
import sys

sys.path.insert(0, "/opt/trn_rl_repo/concourse")
sys.path.insert(0, "/opt/trn_rl_repo")

import ml_dtypes
import numpy as np

import concourse.bass as bass
import concourse.mybir as mybir
import concourse.tile as tile
import bass_rust
from concourse.tile_sem_assignment import N_PROCS

T, B, NCORES = 256, 1024, 8
BC = B // NCORES          # batch per core
H1, H2 = 64, 128
F32 = mybir.dt.float32
BF16 = mybir.dt.bfloat16
SIG = mybir.ActivationFunctionType.Sigmoid

# PSUM/G slot columns: layer-2 block [i2|f2|o2|g2] then L1/3 [i13|f13|o13|g13]
COL_L2 = {0: 0, 1: 128, 2: 256, 3: 384}
COL_13 = {0: 512, 1: 640, 2: 768, 3: 896}
# slot s in [i, f, o, g] order -> row-block index in torch [i, f, g, o] weights
TG = [0, 1, 3, 2]

LAST_RESULTS = None  # BassKernelResults of the most recent run (for test.py)


# ---------------------------------------------------------------- tile patch
def _patched_drain_and_barrier(self, tick_clock, wait_clock):
    # This walrus build rejects instructions carrying more than one sem
    # wait ("Too many sync wait commands") and TileContext's stock tail
    # drain carries one wait per outstanding proc.  Spread them over one
    # SP NoOp per proc; SP program order then makes the drain itself safe
    # with no waits.
    nc = self.nc
    gclock = tick_clock.global_clock
    for p in range(N_PROCS):
        if gclock[p] <= 0:
            continue
        partial = bass_rust.VectorClock()
        partial.require_at_least(p, gclock[p])
        nop = nc.sync.nop(nofuse=True, hint=f"tile_tail_wait_p{p}")
        wait_clock.add_sem_waits(nop.ins, bass_rust.ScopedClock({None: partial}))
    nc.sync.drain()
    nc.all_engine_barrier()
    assert self.sems is not None
    popped = nc._tile_sem_poison_stack.pop()
    assert popped is self._sem_poison
    nc.clear_and_free_semaphores(list(self.sems.allocated().values()))
    nc.all_engine_barrier()


tile.TileContext._drain_and_barrier = _patched_drain_and_barrier


# ---------------------------------------------------------------- builder
def build(t_steps=T, passes=1, l2_bf16=True):
    """Build the SPMD single-core Bass program for t_steps timesteps.

    passes>1 repeats the whole scan (with state re-init) inside one
    kernel — used only to measure HW time by execution-time deltas."""
    nc = bass.Bass("TRN2", target_bir_lowering=False, debug=False)

    # xt blocks of 8 steps: [blk, 0, j*BC:(j+1)*BC] = x_{8blk+j}; [blk,1,:] = 1
    nblk_x = (t_steps + 7) // 8
    xt = nc.declare_dram_parameter("xt", [nblk_x, 2, 8 * BC], F32, isOutput=False)
    w13 = nc.declare_dram_parameter("w13", [98, 512], F32, isOutput=False)
    DT2 = BF16 if l2_bf16 else F32
    w2rec = nc.declare_dram_parameter("w2rec", [128, 512], DT2, isOutput=False)
    w2fold = nc.declare_dram_parameter("w2fold", [65, 512], DT2, isOutput=False)
    w3fold = nc.declare_dram_parameter("w3fold", [128, 512], DT2, isOutput=False)
    m1t = nc.declare_dram_parameter("m1t", [H1, BC], F32, isOutput=False)
    m2t = nc.declare_dram_parameter("m2t", [H2, BC], F32, isOutput=False)
    n_out = (t_steps + 2) // 8 + 2
    h3st = nc.declare_dram_parameter("h3st", [n_out, 8 * BC], F32, isOutput=True)
    # shape-tags the HLO so different `passes` builds don't collide in the
    # XLA executable cache (nc itself is not part of the cache key)
    ptag = nc.declare_dram_parameter("ptag", [1, 8 * passes], F32, isOutput=False)

    nblk = (t_steps + 7) // 8
    with tile.TileContext(nc) as tc:
        with (
            tc.tile_pool(name="const", bufs=1) as cpool,
            tc.tile_pool(name="ring", bufs=1) as ring,
            tc.tile_pool(name="work", bufs=2) as work,
            tc.tile_pool(name="psum", bufs=2, space="PSUM") as pp,
        ):
            # -------- constants
            w13_t = cpool.tile([98, 512], F32, name="w13_t")
            nc.gpsimd.dma_start(w13_t[:], w13[:])
            w2r_t = cpool.tile([128, 512], DT2, name="w2r_t")
            nc.gpsimd.dma_start(w2r_t[:], w2rec[:])
            w2f_t = cpool.tile([65, 512], DT2, name="w2f_t")
            nc.gpsimd.dma_start(w2f_t[:], w2fold[:])
            w3f_t = cpool.tile([128, 512], DT2, name="w3f_t")
            nc.gpsimd.dma_start(w3f_t[:], w3fold[:])
            m1_t = cpool.tile([H1, BC], F32, name="m1_t")
            nc.gpsimd.dma_start(m1_t[:], m1t[:])
            m2_t = cpool.tile([H2, BC], F32, name="m2_t")
            nc.gpsimd.dma_start(m2_t[:], m2t[:])
            ptag_t = cpool.tile([1, 8 * passes], F32, name="ptag_t")
            nc.gpsimd.dma_start(ptag_t[:], ptag[:])

            # -------- state
            # 16-slot mega ring; slot tau%16 is iter tau's L13 matmul rhs.
            # rows 0-63 H1, row 64 H3, 65-95 zero, 96 x_t, 97 ones.
            rg = ring.tile([98, 16 * BC], F32, name="rg")
            h2b = [ring.tile([128, BC], DT2, name=f"h2b_{i}") for i in range(2)]
            l2f = [ring.tile([65, BC], DT2, name=f"l2f_{i}") for i in range(2)]
            l3f = [ring.tile([128, BC], DT2, name=f"l3f_{i}") for i in range(2)]
            Ct = ring.tile([128, 256], F32, name="Ct")

            for p_ in range(passes):
              out_row = 0
              if True:
                nc.vector.memset(rg[0:96, :], 0.0)
                nc.vector.memset(rg[96:98, :], 1.0)
                for j in range(2):
                    nc.vector.memset(h2b[j][:], 0.0)
                    nc.vector.memset(l2f[j][0:64, :], 0.0)
                    nc.vector.memset(l2f[j][64:65, :], 1.0)
                    nc.vector.memset(l3f[j][:], 0.0)
                nc.vector.memset(Ct[:], 0.0)
                # x+ones for iters [0,8) and [8,16)
                for blk in range(min(2, nblk)):
                    nc.sync.dma_start(
                        rg[96:98, blk * 8 * BC : (blk + 1) * 8 * BC], xt[blk]
                    )
              # -------- main wavefront loop
              for tau in range(t_steps + 2):
                  sl = (tau % 16) * BC
                  so = ((tau + 1) % 16) * BC
                  hcur, hnext = h2b[tau % 2], h2b[(tau + 1) % 2]
                  f2cur, f2next = l2f[tau % 2], l2f[(tau + 1) % 2]
                  f3cur, f3next = l3f[tau % 2], l3f[(tau + 1) % 2]

                  if tau % 8 == 0 and tau > 0 and tau + 8 < t_steps:
                      blk, half = (tau + 8) // 8, (((tau + 8) % 16) // 8)
                      nc.sync.dma_start(
                          rg[96:98, half * 8 * BC : (half + 1) * 8 * BC], xt[blk]
                      )

                  psum = pp.tile([128, 1024], F32, name="psum")
                  for s in range(4):
                      nc.tensor.matmul(
                          psum[0:128, COL_13[s] : COL_13[s] + BC],
                          w13_t[:, s * 128 : (s + 1) * 128],
                          rg[0:98, sl : sl + BC],
                          start=True, stop=False,
                      )
                      nc.tensor.matmul(
                          psum[0:128, COL_13[s] : COL_13[s] + BC],
                          w3f_t[:, s * 128 : (s + 1) * 128],
                          f3cur[:],
                          start=False, stop=True,
                      )
                  for s in range(4):
                      nc.tensor.matmul(
                          psum[0:128, COL_L2[s] : COL_L2[s] + BC],
                          w2r_t[:, s * 128 : (s + 1) * 128],
                          hcur[:],
                          start=True, stop=False,
                      )
                      nc.tensor.matmul(
                          psum[0:128, COL_L2[s] : COL_L2[s] + BC],
                          w2f_t[:, s * 128 : (s + 1) * 128],
                          f2cur[:],
                          start=False, stop=True,
                      )

                  # ---- L1/L3 chain (G13 block, parts 0-64)
                  G13 = work.tile([65, 512], F32, name="G13")
                  nc.scalar.activation(G13[:], psum[0:65, 512:1024], SIG)
                  om1 = work.tile([64, 128], F32, name="om1")
                  nc.gpsimd.tensor_mul(om1[:], G13[0:64, 256:384], m1_t[:])
                  v13 = work.tile([65, 128], F32, name="v13")
                  nc.vector.tensor_mul(v13[:], G13[:, 128:256], Ct[0:65, 128:256])
                  u13 = work.tile([65, 128], F32, name="u13")
                  nc.vector.scalar_tensor_tensor(
                      u13[:], G13[:, 384:512], 0.5, G13[:, 0:128],
                      mybir.AluOpType.subtract, mybir.AluOpType.mult,
                  )
                  nc.vector.tensor_add(Ct[0:65, 128:256], v13[:], u13[:])
                  S13 = work.tile([65, 128], F32, name="S13")
                  nc.scalar.activation(S13[:], Ct[0:65, 128:256], SIG, scale=4.0)
                  nc.vector.scalar_tensor_tensor(
                      rg[0:65, so : so + BC], S13[:], 0.5, G13[:, 256:384],
                      mybir.AluOpType.subtract, mybir.AluOpType.mult,
                  )
                  nc.vector.scalar_tensor_tensor(
                      f2next[0:64, :], S13[0:64, :], 0.5, om1[:],
                      mybir.AluOpType.subtract, mybir.AluOpType.mult,
                  )

                  # ---- L2 chain (G2 block)
                  G2 = work.tile([128, 512], F32, name="G2")
                  nc.scalar.activation(G2[:], psum[0:128, 0:512], SIG)
                  om2 = work.tile([128, 128], F32, name="om2")
                  nc.gpsimd.tensor_mul(om2[:], G2[:, 256:384], m2_t[:])
                  v2 = work.tile([128, 128], F32, name="v2")
                  nc.vector.tensor_mul(v2[:], G2[:, 128:256], Ct[:, 0:128])
                  u2 = work.tile([128, 128], F32, name="u2")
                  nc.vector.scalar_tensor_tensor(
                      u2[:], G2[:, 384:512], 0.5, G2[:, 0:128],
                      mybir.AluOpType.subtract, mybir.AluOpType.mult,
                  )
                  nc.vector.tensor_add(Ct[:, 0:128], v2[:], u2[:])
                  S2 = work.tile([128, 128], F32, name="S2")
                  nc.scalar.activation(S2[:], Ct[:, 0:128], SIG, scale=4.0)
                  nc.vector.scalar_tensor_tensor(
                      hnext[:], S2[:], 0.5, G2[:, 256:384],
                      mybir.AluOpType.subtract, mybir.AluOpType.mult,
                  )
                  nc.vector.scalar_tensor_tensor(
                      f3next[:], S2[:], 0.5, om2[:],
                      mybir.AluOpType.subtract, mybir.AluOpType.mult,
                  )

                  if tau % 8 == 6:
                      half = (((tau + 1) % 16) - 7) // 8
                      nc.sync.dma_start(
                          h3st[out_row : out_row + 1, :],
                          rg[64:65, half * 8 * BC : (half + 1) * 8 * BC],
                      )
                      out_row += 1

                  # boundary fix-ups: wipe garbage states before first real use
                  if tau == 0:
                      nc.vector.memset(Ct[:, 0:128], 0.0)          # C2
                      nc.vector.memset(h2b[1][:], 0.0)             # H2
                  if tau == 1:
                      nc.vector.memset(Ct[64:65, 128:256], 0.0)    # C3
                      nc.vector.memset(rg[64:65, 2 * BC : 3 * BC], 0.0)  # H3 slot 2

              # final flush: both halves (tail slots depend on t_steps % 16)
              for half in range(2):
                  nc.sync.dma_start(
                      h3st[out_row : out_row + 1, :],
                      rg[64:65, half * 8 * BC : (half + 1) * 8 * BC],
                  )
                  out_row += 1

    return nc


# ---------------------------------------------------------------- host prep
def pack_weights(Wih1, Whh1, b1, Wih2, Whh2, b2, Wih3, Whh3, b3):
    """Pack/scale weights into the kernel's lhsT layouts (see module doc)."""
    w13 = np.zeros((98, 512), np.float32)
    w2rec = np.zeros((128, 512), np.float32)
    w2fold = np.zeros((65, 512), np.float32)
    w3fold = np.zeros((128, 512), np.float32)
    for s in range(4):
        tg = TG[s]
        gs = 2.0 if s == 3 else 1.0  # sigma(2x) pre-scale for the g slot
        c = s * 128
        # L1 block: rows 0-63 = 2*Whh1^T, row 65 = Wih1, row 66 = b1
        w13[0:64, c : c + 64] = 2.0 * gs * Whh1[tg * 64 : (tg + 1) * 64, :].T
        w13[96, c : c + 64] = gs * Wih1[tg * 64 : (tg + 1) * 64, 0]
        w13[97, c : c + 64] = gs * b1[tg * 64 : (tg + 1) * 64]
        # L3 col 64: row 64 = 2*Whh3, row 97 = b3
        w13[64, c + 64] = 2.0 * gs * Whh3[tg, 0]
        w13[97, c + 64] = gs * b3[tg]
        w3fold[:, c + 64] = 2.0 * gs * Wih3[tg, :]
        # L2
        c2 = s * 128
        w2rec[:, c2 : c2 + 128] = 2.0 * gs * Whh2[tg * 128 : (tg + 1) * 128, :].T
        w2fold[0:64, c2 : c2 + 128] = 2.0 * gs * Wih2[tg * 128 : (tg + 1) * 128, :].T
        w2fold[64, c2 : c2 + 128] = gs * b2[tg * 128 : (tg + 1) * 128]
    return dict(w13=w13, w2rec=w2rec, w2fold=w2fold, w3fold=w3fold)


def make_in_maps(inputs, t_steps=T, passes=1, l2_bf16=True):
    dt2 = ml_dtypes.bfloat16 if l2_bf16 else np.float32
    w = pack_weights(
        inputs["Wih1"], inputs["Whh1"], inputs["b1"],
        inputs["Wih2"], inputs["Whh2"], inputs["b2"],
        inputs["Wih3"], inputs["Whh3"], inputs["b3"],
    )
    for k in ("w2rec", "w2fold", "w3fold"):
        w[k] = w[k].astype(dt2)
    x = np.asarray(inputs["x"], np.float32)
    m1 = np.asarray(inputs["mask1"], np.float32)
    m2 = np.asarray(inputs["mask2"], np.float32)
    in_maps = []
    for c in range(NCORES):
        sl = slice(c * BC, (c + 1) * BC)
        nblk = (t_steps + 7) // 8
        xa = np.zeros((nblk, 2, 8 * BC), np.float32)
        xc = x[:t_steps, sl, 0]  # [t_steps, BC]
        for blk in range(nblk):
            n = min(8, t_steps - blk * 8)
            xa[blk, 0, : n * BC] = xc[blk * 8 : blk * 8 + n].reshape(-1)
        xa[:, 1, :] = 1.0
        in_maps.append({
            "ptag": np.zeros((1, 8 * passes), np.float32),
            "xt": xa,
            "m1t": np.ascontiguousarray(m1[sl, :].T),
            "m2t": np.ascontiguousarray(m2[sl, :].T),
            **{k: v for k, v in w.items()},
        })
    return in_maps


def _split_multi_waits(bir):
    """This walrus build allows at most ONE sem wait per instruction.

    Tile's scheduler attaches as many waits as deps require, so split:
    any instruction with k>1 waits gets k-1 single-wait NoOps inserted
    before it on the same engine (sequencer order preserves semantics)."""
    n = 0
    for f in bir.get("functions", []):
        for bb in f.get("basic_blocks", f.get("blocks", [])):
            insts = bb.get("instructions", [])
            out = []
            for inst in insts:
                si = inst.get("sync_info")
                waits = (si or {}).get("on_wait") or []
                if len(waits) > 1:
                    for w in waits[:-1]:
                        n += 1
                        out.append({
                            "debug": inst.get("debug", 0),
                            "engine": inst["engine"],
                            "ins": [],
                            "name": f"WSPLIT-{n}",
                            "opcode": "NoOp",
                            "outs": [],
                            "sync_info": {"on_update": [], "on_wait": [w]},
                            "text_hint": "wait_split",
                        })
                    si["on_wait"] = [waits[-1]]
                out.append(inst)
            bb["instructions"] = out
    return n


def finalize(nc):
    """Apply the multi-wait split to nc's serialized BIR (idempotent)."""
    import orjson

    if getattr(nc, "_wsplit_done", False):
        return nc
    bir = orjson.loads(nc.to_json_bytes())
    n = _split_multi_waits(bir)
    blob = orjson.dumps(bir)
    nc.to_json_bytes = lambda: blob
    nc._wsplit_done = True
    nc._wsplit_count = n
    return nc


def out_schedule(t_steps=T):
    """Replay the out-DMA emission schedule.

    Returns a list (one entry per h3st row) of 8-tuples: the LSTM step
    whose H3 occupies slot j of that row (-1 if junk)."""
    last_write = [None] * 16     # slot -> iter of last H13 write
    rows = []
    for tau in range(t_steps + 2):
        last_write[(tau + 1) % 16] = tau
        if tau % 8 == 6:
            half = (((tau + 1) % 16) - 7) // 8
            rows.append(tuple(
                (last_write[8 * half + j] - 2)
                if last_write[8 * half + j] is not None else -1
                for j in range(8)
            ))
    for half in range(2):
        rows.append(tuple(
            (last_write[8 * half + j] - 2)
            if last_write[8 * half + j] is not None else -1
            for j in range(8)
        ))
    return rows


_BUILT = {}


def kernel(**inputs) -> np.ndarray:
    global LAST_RESULTS
    from concourse.bass_utils import run_bass_kernel_spmd

    if T not in _BUILT:
        _BUILT[T] = finalize(build(T))
    nc = _BUILT[T]
    in_maps = make_in_maps(inputs, T)
    res = run_bass_kernel_spmd(
        nc, in_maps, list(range(NCORES)),
        trace=bool(os.environ.get("BASS_TRACE")),
    )
    LAST_RESULTS = res
    m3 = np.asarray(inputs["mask3"], np.float32)  # [B, 1]
    sched = out_schedule(T)
    out = np.empty((T, B, 1), np.float32)
    for c in range(NCORES):
        sl = slice(c * BC, (c + 1) * BC)
        h3 = res.results[c]["h3st"]  # [n_out, 8*BC]
        dec = np.empty((T, BC), np.float32)
        for r, steps in enumerate(sched):
            for j, st in enumerate(steps):
                if 0 <= st < T:
                    dec[st] = h3[r, j * BC : (j + 1) * BC]
        # h3 = 2*H3; output = h3 * mask3
        out[:, sl, 0] = 2.0 * dec * m3[sl, 0][None, :]
    return out



# revision 3
# speedup vs baseline: 2.2857x; 2.2857x over previous
"""3-layer LSTM decoder (T=256, B=1024, H=64/128/1) with locked dropout.

Data-parallel over batch: B=1024 -> 128 per core x 8 NeuronCores.
Single fused Bass/Tile kernel per core runs all three layer scans as a
wavefront (iteration tau computes L1 step tau, L2 step tau-1, L3 step
tau-2) so the per-step work of all layers batches into a few wide
engine instructions:

  - 16 matmuls/iter accumulate gate pre-activations into one PSUM tile
    of 8 slots [i2|i13|f2|f13|o2|o13|g2|g13] (2 = layer2; 13 = layer1
    at partitions 0-63 + layer3 at partition 64). Input GEMMs and
    biases are folded in as extra contraction rows (x row, ones row).
  - ONE sigmoid ACT over the whole 1024-wide PSUM rect per iteration.
    tanh is computed as 2*sigmoid(2x)-1: g-gate weights are pre-scaled
    by 2 and the cell state is tracked as C=c/2, hidden as H=h/2, with
    the compensating 2x folded into all weights that consume H.
  - Cell update in 3 packed DVE ops over [128,256]:
      u = (sg-0.5)*si ; v = sf*C ; C = v+u
    then ONE sigmoid ACT S=sigma(4C) (=sigma(2c)) and H=(S-0.5)*so.
  - Locked dropout masks are applied to H with small per-iter DVE ops;
    mask3 (and the final 2x for h3=2*H3) is applied on the host during
    the gather, as is the output assembly.
"""

import os
import sys

sys.path.insert(0, "/opt/trn_rl_repo/concourse")
sys.path.insert(0, "/opt/trn_rl_repo")

import ml_dtypes
import numpy as np

import concourse.bass as bass
import concourse.mybir as mybir
import concourse.tile as tile
import bass_rust
from concourse.tile_sem_assignment import N_PROCS

T, B, NCORES = 256, 1024, 8
BC = B // NCORES          # batch per core
H1, H2 = 64, 128
F32 = mybir.dt.float32
BF16 = mybir.dt.bfloat16
SIG = mybir.ActivationFunctionType.Sigmoid

# PSUM/G slot columns: layer-2 block [i2|f2|o2|g2] then L1/3 [i13|f13|o13|g13]
COL_L2 = {0: 0, 1: 128, 2: 256, 3: 384}
COL_13 = {0: 512, 1: 640, 2: 768, 3: 896}
# slot s in [i, f, o, g] order -> row-block index in torch [i, f, g, o] weights
TG = [0, 1, 3, 2]

LAST_RESULTS = None  # BassKernelResults of the most recent run (for test.py)


# ---------------------------------------------------------------- tile patch
def _patched_drain_and_barrier(self, tick_clock, wait_clock):
    # This walrus build rejects instructions carrying more than one sem
    # wait ("Too many sync wait commands") and TileContext's stock tail
    # drain carries one wait per outstanding proc.  Spread them over one
    # SP NoOp per proc; SP program order then makes the drain itself safe
    # with no waits.
    nc = self.nc
    gclock = tick_clock.global_clock
    for p in range(N_PROCS):
        if gclock[p] <= 0:
            continue
        partial = bass_rust.VectorClock()
        partial.require_at_least(p, gclock[p])
        nop = nc.sync.nop(nofuse=True, hint=f"tile_tail_wait_p{p}")
        wait_clock.add_sem_waits(nop.ins, bass_rust.ScopedClock({None: partial}))
    nc.sync.drain()
    nc.all_engine_barrier()
    assert self.sems is not None
    popped = nc._tile_sem_poison_stack.pop()
    assert popped is self._sem_poison
    nc.clear_and_free_semaphores(list(self.sems.allocated().values()))
    nc.all_engine_barrier()


tile.TileContext._drain_and_barrier = _patched_drain_and_barrier


# ---------------------------------------------------------------- builder
def build(t_steps=T, passes=1, l2_bf16=True):
    """Build the SPMD single-core Bass program for t_steps timesteps.

    passes>1 repeats the whole scan (with state re-init) inside one
    kernel — used only to measure HW time by execution-time deltas."""
    nc = bass.Bass("TRN2", target_bir_lowering=False, debug=False)

    # xt blocks of 8 steps: [blk, 0, j*BC:(j+1)*BC] = x_{8blk+j}; [blk,1,:] = 1
    nblk_x = (t_steps + 7) // 8
    xt = nc.declare_dram_parameter("xt", [nblk_x, 2, 8 * BC], BF16, isOutput=False)
    w13 = nc.declare_dram_parameter("w13", [98, 512], BF16, isOutput=False)
    DT2 = BF16 if l2_bf16 else F32
    w2rec = nc.declare_dram_parameter("w2rec", [128, 512], DT2, isOutput=False)
    w2fold = nc.declare_dram_parameter("w2fold", [65, 512], DT2, isOutput=False)
    w3fold = nc.declare_dram_parameter("w3fold", [128, 512], DT2, isOutput=False)
    m1t = nc.declare_dram_parameter("m1t", [H1, BC], F32, isOutput=False)
    m2t = nc.declare_dram_parameter("m2t", [H2, BC], F32, isOutput=False)
    n_out = (t_steps + 2) // 8 + 2
    h3st = nc.declare_dram_parameter("h3st", [n_out, 8 * BC], BF16, isOutput=True)
    # shape-tags the HLO so different `passes` builds don't collide in the
    # XLA executable cache (nc itself is not part of the cache key)
    ptag = nc.declare_dram_parameter("ptag", [1, 8 * passes], F32, isOutput=False)

    nblk = (t_steps + 7) // 8
    with tile.TileContext(nc) as tc:
        with (
            tc.tile_pool(name="const", bufs=1) as cpool,
            tc.tile_pool(name="ring", bufs=1) as ring,
            tc.tile_pool(name="work", bufs=2) as work,
            tc.tile_pool(name="psum", bufs=2, space="PSUM") as pp,
        ):
            # -------- constants
            w13_t = cpool.tile([98, 512], BF16, name="w13_t")
            nc.gpsimd.dma_start(w13_t[:], w13[:])
            w2r_t = cpool.tile([128, 512], DT2, name="w2r_t")
            nc.gpsimd.dma_start(w2r_t[:], w2rec[:])
            w2f_t = cpool.tile([65, 512], DT2, name="w2f_t")
            nc.gpsimd.dma_start(w2f_t[:], w2fold[:])
            w3f_t = cpool.tile([128, 512], DT2, name="w3f_t")
            nc.gpsimd.dma_start(w3f_t[:], w3fold[:])
            m1_t = cpool.tile([H1, BC], F32, name="m1_t")
            nc.gpsimd.dma_start(m1_t[:], m1t[:])
            m2_t = cpool.tile([H2, BC], F32, name="m2_t")
            nc.gpsimd.dma_start(m2_t[:], m2t[:])
            ptag_t = cpool.tile([1, 8 * passes], F32, name="ptag_t")
            nc.gpsimd.dma_start(ptag_t[:], ptag[:])

            # -------- state
            # 16-slot mega ring; slot tau%16 is iter tau's L13 matmul rhs.
            # rows 0-63 H1, row 64 H3, 65-95 zero, 96 x_t, 97 ones.
            rg = ring.tile([98, 16 * BC], BF16, name="rg")
            h2b = [ring.tile([128, BC], DT2, name=f"h2b_{i}") for i in range(2)]
            l2f = [ring.tile([65, BC], DT2, name=f"l2f_{i}") for i in range(2)]
            l3f = [ring.tile([128, BC], DT2, name=f"l3f_{i}") for i in range(2)]
            Ct = ring.tile([128, 256], F32, name="Ct")

            for p_ in range(passes):
              out_row = 0
              if True:
                nc.vector.memset(rg[0:96, :], 0.0)
                nc.vector.memset(rg[96:98, :], 1.0)
                for j in range(2):
                    nc.vector.memset(h2b[j][:], 0.0)
                    nc.vector.memset(l2f[j][0:64, :], 0.0)
                    nc.vector.memset(l2f[j][64:65, :], 1.0)
                    nc.vector.memset(l3f[j][:], 0.0)
                nc.vector.memset(Ct[:], 0.0)
                # x+ones for iters [0,8) and [8,16)
                for blk in range(min(2, nblk)):
                    nc.sync.dma_start(
                        rg[96:98, blk * 8 * BC : (blk + 1) * 8 * BC], xt[blk]
                    )
              # -------- main wavefront loop
              for tau in range(t_steps + 2):
                  sl = (tau % 16) * BC
                  so = ((tau + 1) % 16) * BC
                  hcur, hnext = h2b[tau % 2], h2b[(tau + 1) % 2]
                  f2cur, f2next = l2f[tau % 2], l2f[(tau + 1) % 2]
                  f3cur, f3next = l3f[tau % 2], l3f[(tau + 1) % 2]

                  if tau % 8 == 0 and tau > 0 and tau + 8 < t_steps:
                      blk, half = (tau + 8) // 8, (((tau + 8) % 16) // 8)
                      nc.sync.dma_start(
                          rg[96:98, half * 8 * BC : (half + 1) * 8 * BC], xt[blk]
                      )

                  psum = pp.tile([128, 1024], F32, name="psum")
                  for s in range(4):
                      nc.tensor.matmul(
                          psum[0:128, COL_13[s] : COL_13[s] + BC],
                          w13_t[:, s * 128 : (s + 1) * 128],
                          rg[0:98, sl : sl + BC],
                          start=True, stop=False,
                      )
                      nc.tensor.matmul(
                          psum[0:128, COL_13[s] : COL_13[s] + BC],
                          w3f_t[:, s * 128 : (s + 1) * 128],
                          f3cur[:],
                          start=False, stop=True,
                      )
                  for s in range(4):
                      nc.tensor.matmul(
                          psum[0:128, COL_L2[s] : COL_L2[s] + BC],
                          w2r_t[:, s * 128 : (s + 1) * 128],
                          hcur[:],
                          start=True, stop=False,
                      )
                      nc.tensor.matmul(
                          psum[0:128, COL_L2[s] : COL_L2[s] + BC],
                          w2f_t[:, s * 128 : (s + 1) * 128],
                          f2cur[:],
                          start=False, stop=True,
                      )

                  # ---- L1/L3 chain (G13 block, parts 0-64)
                  G13 = work.tile([65, 512], F32, name="G13")
                  nc.scalar.activation(G13[:], psum[0:65, 512:1024], SIG)
                  om1 = work.tile([64, 128], F32, name="om1")
                  nc.gpsimd.tensor_mul(om1[:], G13[0:64, 256:384], m1_t[:])
                  v13 = work.tile([65, 128], F32, name="v13")
                  nc.vector.tensor_mul(v13[:], G13[:, 128:256], Ct[0:65, 128:256])
                  u13 = work.tile([65, 128], F32, name="u13")
                  nc.vector.scalar_tensor_tensor(
                      u13[:], G13[:, 384:512], 0.5, G13[:, 0:128],
                      mybir.AluOpType.subtract, mybir.AluOpType.mult,
                  )
                  nc.vector.tensor_add(Ct[0:65, 128:256], v13[:], u13[:])
                  S13 = work.tile([65, 128], F32, name="S13")
                  nc.scalar.activation(S13[:], Ct[0:65, 128:256], SIG, scale=4.0)
                  nc.vector.scalar_tensor_tensor(
                      rg[0:65, so : so + BC], S13[:], 0.5, G13[:, 256:384],
                      mybir.AluOpType.subtract, mybir.AluOpType.mult,
                  )
                  nc.vector.scalar_tensor_tensor(
                      f2next[0:64, :], S13[0:64, :], 0.5, om1[:],
                      mybir.AluOpType.subtract, mybir.AluOpType.mult,
                  )

                  # ---- L2 chain (G2 block)
                  G2 = work.tile([128, 512], F32, name="G2")
                  nc.scalar.activation(G2[:], psum[0:128, 0:512], SIG)
                  om2 = work.tile([128, 128], F32, name="om2")
                  nc.gpsimd.tensor_mul(om2[:], G2[:, 256:384], m2_t[:])
                  v2 = work.tile([128, 128], F32, name="v2")
                  nc.vector.tensor_mul(v2[:], G2[:, 128:256], Ct[:, 0:128])
                  u2 = work.tile([128, 128], F32, name="u2")
                  nc.vector.scalar_tensor_tensor(
                      u2[:], G2[:, 384:512], 0.5, G2[:, 0:128],
                      mybir.AluOpType.subtract, mybir.AluOpType.mult,
                  )
                  nc.vector.tensor_add(Ct[:, 0:128], v2[:], u2[:])
                  S2 = work.tile([128, 128], F32, name="S2")
                  nc.scalar.activation(S2[:], Ct[:, 0:128], SIG, scale=4.0)
                  nc.vector.scalar_tensor_tensor(
                      hnext[:], S2[:], 0.5, G2[:, 256:384],
                      mybir.AluOpType.subtract, mybir.AluOpType.mult,
                  )
                  nc.vector.scalar_tensor_tensor(
                      f3next[:], S2[:], 0.5, om2[:],
                      mybir.AluOpType.subtract, mybir.AluOpType.mult,
                  )

                  if tau % 8 == 6:
                      half = (((tau + 1) % 16) - 7) // 8
                      nc.sync.dma_start(
                          h3st[out_row : out_row + 1, :],
                          rg[64:65, half * 8 * BC : (half + 1) * 8 * BC],
                      )
                      out_row += 1

                  # boundary fix-ups: wipe garbage states before first real use
                  if tau == 0:
                      nc.vector.memset(Ct[:, 0:128], 0.0)          # C2
                      nc.vector.memset(h2b[1][:], 0.0)             # H2
                  if tau == 1:
                      nc.vector.memset(Ct[64:65, 128:256], 0.0)    # C3
                      nc.vector.memset(rg[64:65, 2 * BC : 3 * BC], 0.0)  # H3 slot 2

              # final flush: both halves (tail slots depend on t_steps % 16)
              for half in range(2):
                  nc.sync.dma_start(
                      h3st[out_row : out_row + 1, :],
                      rg[64:65, half * 8 * BC : (half + 1) * 8 * BC],
                  )
                  out_row += 1

    return nc


# ---------------------------------------------------------------- host prep
def pack_weights(Wih1, Whh1, b1, Wih2, Whh2, b2, Wih3, Whh3, b3):
    """Pack/scale weights into the kernel's lhsT layouts (see module doc)."""
    w13 = np.zeros((98, 512), np.float32)
    w2rec = np.zeros((128, 512), np.float32)
    w2fold = np.zeros((65, 512), np.float32)
    w3fold = np.zeros((128, 512), np.float32)
    for s in range(4):
        tg = TG[s]
        gs = 2.0 if s == 3 else 1.0  # sigma(2x) pre-scale for the g slot
        c = s * 128
        # L1 block: rows 0-63 = 2*Whh1^T, row 65 = Wih1, row 66 = b1
        w13[0:64, c : c + 64] = 2.0 * gs * Whh1[tg * 64 : (tg + 1) * 64, :].T
        w13[96, c : c + 64] = gs * Wih1[tg * 64 : (tg + 1) * 64, 0]
        w13[97, c : c + 64] = gs * b1[tg * 64 : (tg + 1) * 64]
        # L3 col 64: row 64 = 2*Whh3, row 97 = b3
        w13[64, c + 64] = 2.0 * gs * Whh3[tg, 0]
        w13[97, c + 64] = gs * b3[tg]
        w3fold[:, c + 64] = 2.0 * gs * Wih3[tg, :]
        # L2
        c2 = s * 128
        w2rec[:, c2 : c2 + 128] = 2.0 * gs * Whh2[tg * 128 : (tg + 1) * 128, :].T
        w2fold[0:64, c2 : c2 + 128] = 2.0 * gs * Wih2[tg * 128 : (tg + 1) * 128, :].T
        w2fold[64, c2 : c2 + 128] = gs * b2[tg * 128 : (tg + 1) * 128]
    return dict(w13=w13, w2rec=w2rec, w2fold=w2fold, w3fold=w3fold)


def make_in_maps(inputs, t_steps=T, passes=1, l2_bf16=True):
    dt2 = ml_dtypes.bfloat16 if l2_bf16 else np.float32
    w = pack_weights(
        inputs["Wih1"], inputs["Whh1"], inputs["b1"],
        inputs["Wih2"], inputs["Whh2"], inputs["b2"],
        inputs["Wih3"], inputs["Whh3"], inputs["b3"],
    )
    for k in ("w2rec", "w2fold", "w3fold", "w13"):
        w[k] = w[k].astype(dt2)
    x = np.asarray(inputs["x"], np.float32)
    m1 = np.asarray(inputs["mask1"], np.float32)
    m2 = np.asarray(inputs["mask2"], np.float32)
    in_maps = []
    for c in range(NCORES):
        sl = slice(c * BC, (c + 1) * BC)
        nblk = (t_steps + 7) // 8
        xa = np.zeros((nblk, 2, 8 * BC), ml_dtypes.bfloat16)
        xc = x[:t_steps, sl, 0]  # [t_steps, BC]
        for blk in range(nblk):
            n = min(8, t_steps - blk * 8)
            xa[blk, 0, : n * BC] = xc[blk * 8 : blk * 8 + n].reshape(-1)
        xa[:, 1, :] = 1.0
        in_maps.append({
            "ptag": np.zeros((1, 8 * passes), np.float32),
            "xt": xa,
            "m1t": np.ascontiguousarray(m1[sl, :].T),
            "m2t": np.ascontiguousarray(m2[sl, :].T),
            **{k: v for k, v in w.items()},
        })
    return in_maps


def _split_multi_waits(bir):
    """This walrus build allows at most ONE sem wait per instruction.

    Tile's scheduler attaches as many waits as deps require, so split:
    any instruction with k>1 waits gets k-1 single-wait NoOps inserted
    before it on the same engine (sequencer order preserves semantics)."""
    n = 0
    for f in bir.get("functions", []):
        for bb in f.get("basic_blocks", f.get("blocks", [])):
            insts = bb.get("instructions", [])
            out = []
            for inst in insts:
                si = inst.get("sync_info")
                waits = (si or {}).get("on_wait") or []
                if len(waits) > 1:
                    for w in waits[:-1]:
                        n += 1
                        out.append({
                            "debug": inst.get("debug", 0),
                            "engine": inst["engine"],
                            "ins": [],
                            "name": f"WSPLIT-{n}",
                            "opcode": "NoOp",
                            "outs": [],
                            "sync_info": {"on_update": [], "on_wait": [w]},
                            "text_hint": "wait_split",
                        })
                    si["on_wait"] = [waits[-1]]
                out.append(inst)
            bb["instructions"] = out
    return n


def finalize(nc):
    """Apply the multi-wait split to nc's serialized BIR (idempotent)."""
    import orjson

    if getattr(nc, "_wsplit_done", False):
        return nc
    bir = orjson.loads(nc.to_json_bytes())
    n = _split_multi_waits(bir)
    blob = orjson.dumps(bir)
    nc.to_json_bytes = lambda: blob
    nc._wsplit_done = True
    nc._wsplit_count = n
    return nc


def out_schedule(t_steps=T):
    """Replay the out-DMA emission schedule.

    Returns a list (one entry per h3st row) of 8-tuples: the LSTM step
    whose H3 occupies slot j of that row (-1 if junk)."""
    last_write = [None] * 16     # slot -> iter of last H13 write
    rows = []
    for tau in range(t_steps + 2):
        last_write[(tau + 1) % 16] = tau
        if tau % 8 == 6:
            half = (((tau + 1) % 16) - 7) // 8
            rows.append(tuple(
                (last_write[8 * half + j] - 2)
                if last_write[8 * half + j] is not None else -1
                for j in range(8)
            ))
    for half in range(2):
        rows.append(tuple(
            (last_write[8 * half + j] - 2)
            if last_write[8 * half + j] is not None else -1
            for j in range(8)
        ))
    return rows


_BUILT = {}


def kernel(**inputs) -> np.ndarray:
    global LAST_RESULTS
    from concourse.bass_utils import run_bass_kernel_spmd

    if T not in _BUILT:
        _BUILT[T] = finalize(build(T))
    nc = _BUILT[T]
    in_maps = make_in_maps(inputs, T)
    res = run_bass_kernel_spmd(
        nc, in_maps, list(range(NCORES)),
        trace=bool(os.environ.get("BASS_TRACE")),
    )
    LAST_RESULTS = res
    m3 = np.asarray(inputs["mask3"], np.float32)  # [B, 1]
    sched = out_schedule(T)
    out = np.empty((T, B, 1), np.float32)
    for c in range(NCORES):
        sl = slice(c * BC, (c + 1) * BC)
        h3 = np.asarray(res.results[c]["h3st"], np.float32)  # [n_out, 8*BC]
        dec = np.empty((T, BC), np.float32)
        for r, steps in enumerate(sched):
            for j, st in enumerate(steps):
                if 0 <= st < T:
                    dec[st] = h3[r, j * BC : (j + 1) * BC]
        # h3 = 2*H3; output = h3 * mask3
        out[:, sl, 0] = 2.0 * dec * m3[sl, 0][None, :]
    return out

